# revision 46
# baseline (speedup 1.0000x reference)
"""Bahdanau additive-attention kernel for one TRN2 chip (8 NeuronCores).

Reference computation (per batch b):
    q      = dec[b] @ w2 + b2 + b1                      # [1, E]
    H      = enc[b] @ w1                                # [S, E]
    scores = tanh(H + q) @ v (+ bv, softmax-invariant)  # [S, 1]
    attn   = softmax(scores over S)
    out[b] = attn @ enc[b]                              # [E]

Sharding: pure data-parallel over batch. 32 batches / 8 cores = 4 per core.
No collectives. Weights replicated. The host passes enc twice: transposed
([b, e, s]) in fp8-e4m3 for the H matmul, and natural layout in bf16 for the
context reduction.

The dominant H matmul runs in fp8 (e4m3) with MatmulPerfMode.DoubleRow: each
PE instruction contracts TWO 128-row k-chunks (lhsT [128,2,M], rhs [128,2,N])
at fp8's double rate - 2x the bf16/fp32r matmul throughput. w1 is pre-scaled
by 64 on the host so its [-1/32, 1/32] entries land in e4m3's normal range;
the 1/64 descale is fused into the ScalarE tanh (tanh(psum/64 + q)).
Quantization puts the end-to-end relative error at ~1.1e-2 (gate: 2e-2);
the fp8 products accumulate exactly in fp32 PSUM so hardware matches the
host-side estimate.

Per-core dataflow (B=4, S=2048, E=1024), working H^T = w1^T @ enc^T so the
tanh bias (q) is a per-partition scalar fused into the ScalarE activation:

  per s-block of 512:
    encT [e-chunk, s]   <- DMA from host-transposed fp8 enc   (8 chunks)
    H^T chunks          <- 32 DoubleRow PE matmuls (w1 stationary)
    tanh(+q, /64)       <- ScalarE, PSUM -> SBUF (fp32r)
    [lagged 1 block]  scores[1, s] += v^T @ tanh  (DVE dot + ones-matmul)
                      exp on ScalarE (+running sums); attn weights to DRAM
                      and back transposed ([s%128, s/128] layout)
    [lagged 2 blocks] ctx[1, E] += attn^T @ enc chunks (DVE, bf16 enc)
  softmax normalization is deferred to one final scale by 1/sum(exp):
  scores are bounded (|tanh|<1, v fixed) so no max-subtraction is needed.

The one-block lag of the v/exp stage and two-block lag of the context stage
keep the PE stream dense: each stage's inputs (ScalarE tanh, the attn DRAM
roundtrip) are ready long before the PE reaches it.
"""

import os
import sys

sys.path.insert(0, "/opt/trn_rl_repo")

import numpy as np  # noqa: E402

import concourse.tile as tile  # noqa: E402
from concourse import bacc, mybir  # noqa: E402
from concourse.bass import ts  # noqa: E402
from concourse.bass_utils import run_bass_kernel_spmd  # noqa: E402

P = 128
N_CORES = 8
B_TOTAL = 32
B = B_TOTAL // N_CORES  # 4 batches per core
S = 2048
E = 1024
EC = E // P  # 8 chunks of the hidden dim
EC2 = EC // 2  # 4 double-chunks (DoubleRow pairs)
SB = 512  # s-block (matmul moving size)
NSB = S // SB  # 4 s-blocks per batch
SK = S // P  # 16 s-chunks of 128 per batch
KSB = SB // P  # 4 s-chunks per s-block

F32 = mybir.dt.float32
F32R = mybir.dt.float32r
BF16 = mybir.dt.bfloat16
F8 = mybir.dt.float8e4  # e4m3
U8 = mybir.dt.uint8  # fp8 bytes travel as uint8: the fp8-typed host->device
# upload path corrupts part of the array; same bytes as uint8 arrive intact

W1_SCALE = 64.0  # host multiplies w1 by this before fp8 quantization

SD = F32R  # storage dtype of the DVE-side dataflow (bitcast f32)
Act = mybir.ActivationFunctionType
DR = mybir.MatmulPerfMode.DoubleRow

# bisection switches (temporary): set to "f32r" to revert a piece to baseline
Q_DT = F32R if os.environ.get("ATTN_Q") == "f32r" else BF16
CENC_DT = F32R if os.environ.get("ATTN_CENC") == "f32r" else BF16
H_FP8 = os.environ.get("ATTN_H") != "f32r"
WARM_GROUPS = int(os.environ.get("ATTN_WARM", "0"))


def _f32(ap):
    return ap if ap.dtype is F32 else ap.bitcast(F32)


DEBUG = os.environ.get("ATTN_DEBUG") == "1"


def _build_body(nc, tc, ctx, enc, encT_d, dec, w1, b1, w2, b2, v, out, dbg):
    # ---------------- persistent constants ----------------
    const = ctx.enter_context(tc.tile_pool(name="const", bufs=1))
    dram = ctx.enter_context(tc.tile_pool(name="dram", bufs=2, space="DRAM"))

    qT = const.tile([P, EC, B], F32)  # [p, c, b] = q_full[b, c*128+p]
    ones_f = const.tile([P, 1], F32)
    ones_sd = const.tile([P, 1], SD, name="ones_sd")
    ones_b = const.tile([P, 1], BF16, name="ones_b")
    nc.vector.memset(ones_f[:], 1.0)
    nc.vector.tensor_copy(ones_sd[:], ones_f[:])
    nc.vector.memset(ones_b[:], 1.0)

    # ---------------- main pools ----------------
    # Created BEFORE the setup pool: the first encT DMA must not land in a
    # region previously touched by the setup tiles / the 4-byte-stride qT
    # gather DMAs -- on HW that combination deterministically truncated the
    # low mantissa bits of the first encT tile (reduced-precision DMA path).
    encT_pool = ctx.enter_context(tc.tile_pool(name="encT", bufs=2))
    cenc_pool = ctx.enter_context(tc.tile_pool(name="cenc", bufs=13))
    work = ctx.enter_context(tc.tile_pool(name="work", bufs=18))
    accp = ctx.enter_context(tc.tile_pool(name="accp", bufs=2))
    onep = ctx.enter_context(tc.tile_pool(name="onep", bufs=2))
    ps_h = ctx.enter_context(tc.tile_pool(name="ps_h", bufs=3, space="PSUM"))
    ps_s = ctx.enter_context(tc.tile_pool(name="ps_s", bufs=2, space="PSUM"))
    ps_d = ctx.enter_context(tc.tile_pool(name="ps_d", bufs=1, space="PSUM"))
    ps_c = ctx.enter_context(tc.tile_pool(name="ps_c", bufs=1, space="PSUM"))

    def encT_dma(b, sb):
        encT = encT_pool.tile([P, EC, SB], F8 if H_FP8 else SD, tag="encT")
        encT_ap = encT_d[:].bitcast(F8) if H_FP8 else encT_d[:]
        encT_r = encT_ap[b].rearrange("(c p) s -> p c s", p=P)
        for c in range(EC):
            nc.sync.dma_start(encT[:, c, :], encT_r[:, c, ts(sb, SB)])
        return encT

    # ---- setup (scoped): ----
    # q = dec @ w2 (PE, bf16) -> DRAM roundtrip into [p, c, b] layout, + b1+b2.
    # q's inputs stream first so the PE's opening matmuls aren't starved;
    # no ScalarE ops here (the first tanh, which reads qT, is behind these
    # producers in program order).
    with (
        tc.tile_pool(name="setup", bufs=1) as setup,
        tc.tile_pool(name="setup_ps", bufs=1, space="PSUM") as setup_ps,
    ):
        w2_sb = setup.tile([P, EC, E], Q_DT)
        w2_r = w2[:].rearrange("(c p) e -> p c e", p=P)
        decT = setup.tile([P, EC, B], Q_DT)  # [p, c, b] = dec[b, 0, c*128+p]
        dec_r = dec[:][:, 0, :].rearrange("b (c p) -> p c b", p=P)
        for c in range(EC):
            nc.sync.dma_start(decT[:, c, :], dec_r[:, c, :])
        b12T = setup.tile([P, EC], F32)
        b1_sb = setup.tile([P, EC], F32)
        b2_sb = setup.tile([P, EC], F32)
        nc.sync.dma_start(b1_sb[:], b1[:].rearrange("(c p) -> p c", p=P))
        nc.sync.dma_start(b2_sb[:], b2[:].rearrange("(c p) -> p c", p=P))
        nc.vector.tensor_add(b12T[:], b1_sb[:], b2_sb[:])

        w1_sb = const.tile([P, EC, E], F8 if H_FP8 else SD)  # w1[c*128+p, e']
        w1_ap = w1[:].bitcast(F8) if H_FP8 else w1[:]
        w1_r = w1_ap.rearrange("(c p) e -> p c e", p=P)
        # interleaved so q's w2 chunks and the first mains' w1 stream together
        for c in range(EC):
            nc.sync.dma_start(w2_sb[:, c, :], w2_r[:, c, :])
            nc.sync.dma_start(w1_sb[:, c, :], w1_r[:, c, :])
        vT = const.tile([P, EC], SD)  # [p, c] = v[c*128+p, 0]
        nc.sync.dma_start(vT[:], v[:][:, 0].rearrange("(c p) -> p c", p=P))

        # prefetch the first s-block's encT ahead of the q/qT DMAs (see the
        # main-pool comment: ordering after them corrupts this tile on HW)
        encT_first = encT_dma(0, 0)

        # q computed directly in [e'-partition, b] layout: stationary w2
        # chunk, moving decT columns -> PSUM [128, B]; bias add fuses b1+b2.
        # No DRAM roundtrip (whose 4-byte-stride gather DMAs also sat right
        # where the encT-corrupting DMA window was).
        for cp in range(EC):
            q_ps = setup_ps.tile([P, B], F32, tag="q_ps")
            for c in range(EC):
                nc.tensor.matmul(
                    q_ps[:],
                    w2_sb[:, c, ts(cp, P)],
                    decT[:, c, :],
                    start=(c == 0),
                    stop=(c == EC - 1),
                )
            nc.vector.tensor_scalar_add(
                qT[:, cp, :], q_ps[:], b12T[:, cp : cp + 1]
            )
        if DEBUG:
            nc.sync.dma_start(dbg["qT"][:], qT[:])

    # PE warm-up: on HW the first fp8-DoubleRow window after the f32r/bf16
    # q matmuls computes corrupted PSUM (first-s-block-of-batch-0 signature;
    # later identical instructions are fine). Burn that window on dummy
    # DoubleRow groups whose results are discarded (a token column is DMA'd
    # out so the instructions aren't dead-code-eliminated).
    if H_FP8 and WARM_GROUPS > 0:
        warm_sb = const.tile([P, WARM_GROUPS], F32, name="warm_sb")
        for g in range(WARM_GROUPS):
            wps = ps_h.tile([P, SB], F32, tag="ph")
            for c2 in range(EC2):
                nc.tensor.matmul(
                    wps[:],
                    w1_sb[:, 2 * c2 : 2 * c2 + 2, 0:P],
                    w1_sb[:, 2 * c2 : 2 * c2 + 2, 0:SB],
                    start=(c2 == 0),
                    stop=(c2 == EC2 - 1),
                    perf_mode=DR,
                )
            nc.vector.tensor_copy(warm_sb[:, g : g + 1], wps[:, 0:1])
        warm_d = dram.tile([P, WARM_GROUPS], F32, tag="warm_d")
        nc.sync.dma_start(warm_d[:], warm_sb[:])

    # Work deferred so the PE never waits on ScalarE output or DMA
    # roundtrips: flushed one (v/exp) or two (ctx) s-blocks later.
    pending_v = []
    pending_ctx = []

    def flush_one(queue):
        if queue:
            queue.pop(0)()

    for b in range(B):
        sumsT = onep.tile([P, NSB], F32, tag="sums")
        expT = work.tile([P, SK], SD, tag="expT")  # [p, k] = exp[k*128+p]
        recip = onep.tile([1, 1], F32, tag="recip")
        cstate = {}  # running DVE accumulator for the context reduction

        for sb in range(NSB):
            # encT[p, c, j] = enc[b, sb*512+j, c*128+p], from host transpose
            if b == 0 and sb == 0:
                encT = encT_first
            else:
                encT = encT_dma(b, sb)
            # ---- main matmuls: H^T chunks via fp8 DoubleRow, tanh(+q) ----
            # Each DoubleRow instruction contracts e-chunks (2*c2, 2*c2+1):
            # lhsT [128, 2, 128] and rhs [128, 2, 512] pair along dim 1.
            ths = []
            for cp in range(EC):
                ph = ps_h.tile([P, SB], F32, tag="ph")
                if H_FP8:
                    for c2 in range(EC2):
                        nc.tensor.matmul(
                            ph[:],
                            w1_sb[:, 2 * c2 : 2 * c2 + 2, ts(cp, P)],
                            encT[:, 2 * c2 : 2 * c2 + 2, :],
                            start=(c2 == 0),
                            stop=(c2 == EC2 - 1),
                            perf_mode=DR,
                        )
                else:
                    for c in range(EC):
                        nc.tensor.matmul(
                            ph[:],
                            w1_sb[:, c, ts(cp, P)],
                            encT[:, c, :],
                            start=(c == 0),
                            stop=(c == EC - 1),
                        )
                th = work.tile([P, SB], BF16, tag="tanh")
                nc.scalar.activation(
                    th[:],
                    ph[:],
                    Act.Tanh,
                    bias=qT[:, cp, b : b + 1],
                    scale=(1.0 / W1_SCALE) if H_FP8 else 1.0,
                )
                if DEBUG:
                    key = (b, sb, cp)
                    slot = {(0, 0, 0): 0, (0, 0, 7): 1, (0, 1, 0): 2,
                            (1, 0, 0): 3}.get(key)
                    if slot is not None:
                        nc.sync.dma_start(dbg["th"][:][slot], _f32(th[:]))
                    if key == (0, 0, 0):
                        phc = work.tile([P, SB], F32, tag="dbg_ph")
                        nc.vector.tensor_copy(phc[:], ph[:])
                        nc.sync.dma_start(dbg["ph"][:], phc[:])
                ths.append(th)

            if DEBUG and b == 0 and sb <= 1:
                nc.sync.dma_start(dbg["encT"][:][sb], encT[:])

            # prefetch the natural-layout bf16 enc chunks this block's
            # (2-block lagged) ctx reduction will need; issued after the
            # mains so they stay off the startup-critical DMA window
            cencs = []
            enc_b = enc[:][b].rearrange("(k p) e -> p k e", p=P)
            for k in range(sb * KSB, (sb + 1) * KSB):
                ce = cenc_pool.tile([P, E], CENC_DT, tag="cenc")
                nc.sync.dma_start(ce[:], enc_b[:, k, :])
                cencs.append(ce)

            flush_one(pending_v)
            if len(pending_ctx) >= 2:
                flush_one(pending_ctx)

            def make_v(
                b=b,
                sb=sb,
                ths=ths,
                sumsT=sumsT,
                expT=expT,
                recip=recip,
            ):
                def issue():
                    # acc[p, s] = sum_cp tanh_cp[p, s] * v_cp[p]  (VectorE,
                    # ping-pong accumulator), then per 128-wide s-chunk one
                    # tiny matmul acc[:, chunk].T @ ones -> scoresT column
                    # [128, 1]: scores land already transposed, so exp can
                    # write expT directly in SBUF (no DRAM roundtrip)
                    acc = None
                    for cp in range(EC):
                        nxt = accp.tile(
                            [P, SB], BF16, tag=f"vacc{cp % 2}", name="vacc"
                        )
                        if acc is None:
                            nc.vector.tensor_scalar_mul(
                                nxt[:], ths[cp][:], _f32(vT[:, cp : cp + 1])
                            )
                        else:
                            nc.vector.scalar_tensor_tensor(
                                nxt[:],
                                ths[cp][:],
                                _f32(vT[:, cp : cp + 1]),
                                acc[:],
                                mybir.AluOpType.mult,
                                mybir.AluOpType.add,
                            )
                        acc = nxt
                    psT = ps_s.tile([P, KSB], F32, tag="pssT", name="pssT")
                    for j in range(KSB):
                        nc.tensor.matmul(
                            psT[:, j : j + 1],
                            acc[:, ts(j, P)],
                            ones_b[:],
                            start=True,
                            stop=True,
                        )
                    # exp + per-partition running sums
                    # (no max needed: |scores| <= 32)
                    nc.scalar.activation(
                        expT[:, ts(sb, KSB)],
                        psT[:],
                        Act.Exp,
                        accum_out=sumsT[:, sb : sb + 1],
                    )
                    if DEBUG and sb == NSB - 1 and b <= 1:
                        nc.sync.dma_start(dbg["expT"][:][b], _f32(expT[:]))
                    if sb == NSB - 1:
                        # softmax denominator: partition-sum of sumsT via a
                        # small fp32 ones-matmul, then reduce + reciprocal.
                        # Must be issued AFTER the final sumsT write (Tile
                        # deps follow program order).
                        psd = ps_d.tile([1, NSB], F32, tag="psd", name="psd")
                        nc.tensor.matmul(
                            psd[:], ones_f[:], sumsT[:], start=True, stop=True
                        )
                        ssum = onep.tile([1, 1], F32, tag="ssum", name="ssum")
                        nc.vector.tensor_reduce(
                            ssum[:],
                            psd[:],
                            mybir.AxisListType.X,
                            mybir.AluOpType.add,
                        )
                        nc.vector.reciprocal(recip[:], ssum[:])

                return issue

            def make_ctx(
                b=b,
                sb=sb,
                cencs=cencs,
                expT=expT,
                cstate=cstate,
                recip=recip,
                last=(sb == NSB - 1),
            ):
                def issue():
                    # acc2[p, e] += enc[k*128+p, e] * attn[k*128+p] (VectorE)
                    # bf16 sub-chains (2x DVE rate) merged into an f32 main
                    # accumulator every 2 s-blocks - the 8-term bf16 partials
                    # stay tiny so their rounding is negligible;
                    # partition-sum via ones-matmul at the end
                    for j, k in enumerate(range(sb * KSB, (sb + 1) * KSB)):
                        nxt = accp.tile(
                            [P, E], BF16, tag=f"csub{k % 2}", name="csub"
                        )
                        attn_k = _f32(expT[:, k : k + 1])
                        if k % (2 * KSB) == 0:
                            nc.vector.tensor_scalar_mul(
                                nxt[:], cencs[j][:], attn_k
                            )
                        else:
                            nc.vector.scalar_tensor_tensor(
                                nxt[:],
                                cencs[j][:],
                                attn_k,
                                cstate["sub"][:],
                                mybir.AluOpType.mult,
                                mybir.AluOpType.add,
                            )
                        cstate["sub"] = nxt
                    if sb % 2 == 1:
                        mrg = accp.tile(
                            [P, E], SD, tag=f"cacc{(sb // 2) % 2}", name="cacc"
                        )
                        if sb == 1:
                            nc.vector.tensor_copy(mrg[:], cstate["sub"][:])
                        else:
                            nc.vector.tensor_add(
                                mrg[:], cstate["sub"][:], cstate["main"][:]
                            )
                        cstate["main"] = mrg
                    if last:
                        acc2 = cstate["main"]
                        for h in range(E // SB):
                            psc = ps_c.tile(
                                [1, SB], F32, tag="psc", name="psc"
                            )
                            nc.tensor.matmul(
                                psc[:],
                                ones_sd[:],
                                acc2[:, ts(h, SB)],
                                start=True,
                                stop=True,
                            )
                            ctx_sb = onep.tile(
                                [1, SB], F32, tag="ctx", name="ctx_sb"
                            )
                            nc.scalar.activation(
                                ctx_sb[:], psc[:], Act.Copy, scale=recip[:]
                            )
                            nc.sync.dma_start(
                                out[:][b : b + 1, ts(h, SB)], ctx_sb[:]
                            )

                return issue

            pending_v.append(make_v())
            pending_ctx.append(make_ctx())

    while pending_v or pending_ctx:
        flush_one(pending_v)
        flush_one(pending_ctx)


def build_nc():
    nc = bacc.Bacc(
        "TRN2", target_bir_lowering=False, debug=False, num_devices=N_CORES
    )
    enc = nc.dram_tensor("encoder_outputs", [B, S, E], CENC_DT, kind="ExternalInput")
    encT_d = nc.dram_tensor(
        "encoder_outputs_t", [B, E, S], U8 if H_FP8 else SD, kind="ExternalInput"
    )
    dec = nc.dram_tensor("decoder_output", [B, 1, E], Q_DT, kind="ExternalInput")
    w1 = nc.dram_tensor("w1", [E, E], U8 if H_FP8 else SD, kind="ExternalInput")
    b1 = nc.dram_tensor("b1", [E], F32, kind="ExternalInput")
    w2 = nc.dram_tensor("w2", [E, E], Q_DT, kind="ExternalInput")
    b2 = nc.dram_tensor("b2", [E], F32, kind="ExternalInput")
    v = nc.dram_tensor("v", [E, 1], SD, kind="ExternalInput")
    out = nc.dram_tensor("out", [B, E], F32, kind="ExternalOutput")
    dbg = {}
    if DEBUG:
        dbg["qT"] = nc.dram_tensor("dbg_qT", [P, EC, B], F32, kind="ExternalOutput")
        dbg["th"] = nc.dram_tensor("dbg_th", [4, P, SB], F32, kind="ExternalOutput")
        dbg["ph"] = nc.dram_tensor("dbg_ph", [P, SB], F32, kind="ExternalOutput")
        dbg["expT"] = nc.dram_tensor("dbg_expT", [2, P, SK], F32, kind="ExternalOutput")
        dbg["encT"] = nc.dram_tensor(
            "dbg_encT", [2, P, EC, SB], F8 if H_FP8 else SD,
            kind="ExternalOutput"
        )

    from contextlib import ExitStack

    with tile.TileContext(nc) as tc:
        with ExitStack() as ctx:
            _build_body(nc, tc, ctx, enc, encT_d, dec, w1, b1, w2, b2, v, out, dbg)
    nc.compile()
    return nc


_NC_CACHE = None


def _get_nc():
    global _NC_CACHE
    if _NC_CACHE is None:
        _NC_CACHE = build_nc()
    return _NC_CACHE


def make_in_maps(inputs):
    """Host-side prep: shard over batch, quantize (fp8 transposed enc for the
    H matmul, bf16 natural enc for the context stage, fp8 w1 scaled by 64)."""
    f32 = np.float32
    q_np = mybir.dt.np(Q_DT)
    cenc_np = mybir.dt.np(CENC_DT)
    h_np = mybir.dt.np(F8) if H_FP8 else f32
    enc_all = np.asarray(inputs["encoder_outputs"], dtype=f32)
    enc_bf16 = np.ascontiguousarray(enc_all.astype(cenc_np))
    encT_f8 = np.ascontiguousarray(
        enc_all.astype(h_np).transpose(0, 2, 1)
    )
    dec_bf16 = np.asarray(inputs["decoder_output"], dtype=f32).astype(q_np)
    w1_f32 = np.asarray(inputs["w1"], dtype=f32)
    w1_f8 = (w1_f32 * f32(W1_SCALE)).astype(h_np) if H_FP8 else w1_f32
    if H_FP8:
        encT_f8 = encT_f8.view(np.uint8)
        w1_f8 = w1_f8.view(np.uint8)
    w2_bf16 = np.asarray(inputs["w2"], dtype=f32).astype(q_np)
    in_maps = []
    for i in range(N_CORES):
        sl = slice(i * B, (i + 1) * B)
        in_maps.append(
            {
                "encoder_outputs": np.ascontiguousarray(enc_bf16[sl]),
                "encoder_outputs_t": encT_f8[sl],
                "decoder_output": np.ascontiguousarray(dec_bf16[sl]),
                "w1": w1_f8,
                "b1": np.ascontiguousarray(inputs["b1"], dtype=f32),
                "w2": w2_bf16,
                "b2": np.ascontiguousarray(inputs["b2"], dtype=f32),
                "v": np.ascontiguousarray(inputs["v"], dtype=f32),
            }
        )
    return in_maps


def run(inputs, trace=False):
    """Run on hardware. Returns (output [32, 1024] f32, exec_time_ns or None)."""
    nc = _get_nc()
    in_maps = make_in_maps(inputs)
    res = run_bass_kernel_spmd(
        nc, in_maps, core_ids=list(range(N_CORES)), trace=trace
    )
    out = np.concatenate([np.asarray(r["out"]) for r in res.results], axis=0)
    return out, res.exec_time_ns


def kernel(**inputs):
    out, _ = run(inputs)
    return out


# revision 54
# speedup vs baseline: 1.0926x; 1.0926x over previous
"""Bahdanau additive-attention kernel for one TRN2 chip (8 NeuronCores).

Reference computation (per batch b):
    q      = dec[b] @ w2 + b2 + b1                      # [1, E]
    H      = enc[b] @ w1                                # [S, E]
    scores = tanh(H + q) @ v (+ bv, softmax-invariant)  # [S, 1]
    attn   = softmax(scores over S)
    out[b] = attn @ enc[b]                              # [E]

Sharding: pure data-parallel over batch. 32 batches / 8 cores = 4 per core.
No collectives. Weights replicated. The host passes enc twice: transposed
([b, e, s]) in fp8-e4m3 for the H matmul, and natural layout in bf16 for the
context reduction.

The dominant H matmul runs in fp8 (e4m3) with MatmulPerfMode.DoubleRow: each
PE instruction contracts TWO 128-row k-chunks (lhsT [128,2,M], rhs [128,2,N])
at fp8's double rate - 2x the bf16/fp32r matmul throughput. w1 is pre-scaled
by 64 on the host so its [-1/32, 1/32] entries land in e4m3's normal range;
the 1/64 descale is fused into the ScalarE tanh (tanh(psum/64 + q)).
Quantization puts the end-to-end relative error at ~1.1e-2 (gate: 2e-2);
the fp8 products accumulate exactly in fp32 PSUM so hardware matches the
host-side estimate.

Per-core dataflow (B=4, S=2048, E=1024), working H^T = w1^T @ enc^T so the
tanh bias (q) is a per-partition scalar fused into the ScalarE activation:

  per s-block of 512:
    encT [e-chunk, s]   <- DMA from host-transposed fp8 enc   (8 chunks)
    H^T chunks          <- 32 DoubleRow PE matmuls (w1 stationary)
    tanh(+q, /64)       <- ScalarE, PSUM -> SBUF (fp32r)
    [lagged 1 block]  scores[1, s] += v^T @ tanh  (DVE dot + ones-matmul)
                      exp on ScalarE (+running sums); attn weights to DRAM
                      and back transposed ([s%128, s/128] layout)
    [lagged 2 blocks] ctx[1, E] += attn^T @ enc chunks (DVE, bf16 enc)
  softmax normalization is deferred to one final scale by 1/sum(exp):
  scores are bounded (|tanh|<1, v fixed) so no max-subtraction is needed.

The one-block lag of the v/exp stage and two-block lag of the context stage
keep the PE stream dense: each stage's inputs (ScalarE tanh, the attn DRAM
roundtrip) are ready long before the PE reaches it.
"""

import os
import sys

sys.path.insert(0, "/opt/trn_rl_repo")

import numpy as np  # noqa: E402

import concourse.tile as tile  # noqa: E402
from concourse import bacc, mybir  # noqa: E402
from concourse.bass import ts  # noqa: E402
from concourse.bass_utils import run_bass_kernel_spmd  # noqa: E402

P = 128
N_CORES = 8
B_TOTAL = 32
B = B_TOTAL // N_CORES  # 4 batches per core
S = 2048
E = 1024
EC = E // P  # 8 chunks of the hidden dim
EC2 = EC // 2  # 4 double-chunks (DoubleRow pairs)
SB = 512  # s-block (matmul moving size)
NSB = S // SB  # 4 s-blocks per batch
SK = S // P  # 16 s-chunks of 128 per batch
KSB = SB // P  # 4 s-chunks per s-block

F32 = mybir.dt.float32
F32R = mybir.dt.float32r
BF16 = mybir.dt.bfloat16
F8 = mybir.dt.float8e4  # e4m3
U8 = mybir.dt.uint8  # fp8 bytes travel as uint8: the fp8-typed host->device
# upload path corrupts part of the array; same bytes as uint8 arrive intact

W1_SCALE = 64.0  # host multiplies w1 by this before fp8 quantization

SD = F32R  # storage dtype of the DVE-side dataflow (bitcast f32)
Act = mybir.ActivationFunctionType
DR = mybir.MatmulPerfMode.DoubleRow

# bisection switches (temporary): set to "f32r" to revert a piece to baseline
Q_DT = F32R if os.environ.get("ATTN_Q") == "f32r" else BF16
CENC_DT = F32R if os.environ.get("ATTN_CENC") == "f32r" else BF16
H_FP8 = os.environ.get("ATTN_H") != "f32r"
WARM_GROUPS = int(os.environ.get("ATTN_WARM", "0"))


def _f32(ap):
    return ap if ap.dtype is F32 else ap.bitcast(F32)


DEBUG = os.environ.get("ATTN_DEBUG") == "1"


def _build_body(nc, tc, ctx, enc, encT_d, dec, w1, b1, w2, b2, v, out, dbg):
    # ---------------- persistent constants ----------------
    const = ctx.enter_context(tc.tile_pool(name="const", bufs=1))
    dram = ctx.enter_context(tc.tile_pool(name="dram", bufs=2, space="DRAM"))

    qT = const.tile([P, EC, B], F32)  # [p, c, b] = q_full[b, c*128+p]
    ones_f = const.tile([P, 1], F32)
    ones_sd = const.tile([P, 1], SD, name="ones_sd")
    ones_b = const.tile([P, 1], BF16, name="ones_b")
    nc.vector.memset(ones_f[:], 1.0)
    nc.vector.tensor_copy(ones_sd[:], ones_f[:])
    nc.vector.memset(ones_b[:], 1.0)

    # ---------------- main pools ----------------
    # Created BEFORE the setup pool: the first encT DMA must not land in a
    # region previously touched by the setup tiles / the 4-byte-stride qT
    # gather DMAs -- on HW that combination deterministically truncated the
    # low mantissa bits of the first encT tile (reduced-precision DMA path).
    encT_pool = ctx.enter_context(tc.tile_pool(name="encT", bufs=2))
    cenc_pool = ctx.enter_context(tc.tile_pool(name="cenc", bufs=13))
    work = ctx.enter_context(tc.tile_pool(name="work", bufs=18))
    accp = ctx.enter_context(tc.tile_pool(name="accp", bufs=2))
    onep = ctx.enter_context(tc.tile_pool(name="onep", bufs=2))
    ps_h = ctx.enter_context(tc.tile_pool(name="ps_h", bufs=3, space="PSUM"))
    ps_s = ctx.enter_context(tc.tile_pool(name="ps_s", bufs=2, space="PSUM"))
    ps_c = ctx.enter_context(tc.tile_pool(name="ps_c", bufs=1, space="PSUM"))

    def encT_dma(b, sb):
        encT = encT_pool.tile([P, EC, SB], F8 if H_FP8 else SD, tag="encT")
        encT_ap = encT_d[:].bitcast(F8) if H_FP8 else encT_d[:]
        encT_r = encT_ap[b].rearrange("(c p) s -> p c s", p=P)
        for c in range(EC):
            nc.sync.dma_start(encT[:, c, :], encT_r[:, c, ts(sb, SB)])
        return encT

    # ---- setup (scoped): ----
    # q = dec @ w2 (PE, bf16) -> DRAM roundtrip into [p, c, b] layout, + b1+b2.
    # q's inputs stream first so the PE's opening matmuls aren't starved;
    # no ScalarE ops here (the first tanh, which reads qT, is behind these
    # producers in program order).
    with (
        tc.tile_pool(name="setup", bufs=1) as setup,
        tc.tile_pool(name="setup_ps", bufs=1, space="PSUM") as setup_ps,
    ):
        w2_sb = setup.tile([P, EC, E], Q_DT)
        w2_r = w2[:].rearrange("(c p) e -> p c e", p=P)
        decT = setup.tile([P, EC, B], Q_DT)  # [p, c, b] = dec[b, 0, c*128+p]
        dec_r = dec[:][:, 0, :].rearrange("b (c p) -> p c b", p=P)
        for c in range(EC):
            nc.sync.dma_start(decT[:, c, :], dec_r[:, c, :])
        b12T = setup.tile([P, EC], F32)
        b1_sb = setup.tile([P, EC], F32)
        b2_sb = setup.tile([P, EC], F32)
        nc.sync.dma_start(b1_sb[:], b1[:].rearrange("(c p) -> p c", p=P))
        nc.sync.dma_start(b2_sb[:], b2[:].rearrange("(c p) -> p c", p=P))
        nc.vector.tensor_add(b12T[:], b1_sb[:], b2_sb[:])

        w1_sb = const.tile([P, EC, E], F8 if H_FP8 else SD)  # w1[c*128+p, e']
        w1_ap = w1[:].bitcast(F8) if H_FP8 else w1[:]
        w1_r = w1_ap.rearrange("(c p) e -> p c e", p=P)
        # interleaved so q's w2 chunks and the first mains' w1 stream together
        for c in range(EC):
            nc.sync.dma_start(w2_sb[:, c, :], w2_r[:, c, :])
            nc.sync.dma_start(w1_sb[:, c, :], w1_r[:, c, :])
        vT = const.tile([P, EC], SD)  # [p, c] = v[c*128+p, 0]
        nc.sync.dma_start(vT[:], v[:][:, 0].rearrange("(c p) -> p c", p=P))
        vT_b = const.tile([P, EC], BF16, name="vT_b")  # v-matmul stationary
        nc.vector.tensor_copy(vT_b[:], _f32(vT[:]))

        # prefetch the first s-block's encT ahead of the q/qT DMAs (see the
        # main-pool comment: ordering after them corrupts this tile on HW)
        encT_first = encT_dma(0, 0)

        # q computed directly in [e'-partition, b] layout: stationary w2
        # chunk, moving decT columns -> PSUM [128, B]; bias add fuses b1+b2.
        # No DRAM roundtrip (whose 4-byte-stride gather DMAs also sat right
        # where the encT-corrupting DMA window was).
        for cp in range(EC):
            q_ps = setup_ps.tile([P, B], F32, tag="q_ps")
            for c in range(EC):
                nc.tensor.matmul(
                    q_ps[:],
                    w2_sb[:, c, ts(cp, P)],
                    decT[:, c, :],
                    start=(c == 0),
                    stop=(c == EC - 1),
                )
            nc.vector.tensor_scalar_add(
                qT[:, cp, :], q_ps[:], b12T[:, cp : cp + 1]
            )
        if DEBUG:
            nc.sync.dma_start(dbg["qT"][:], qT[:])

    # PE warm-up: on HW the first fp8-DoubleRow window after the f32r/bf16
    # q matmuls computes corrupted PSUM (first-s-block-of-batch-0 signature;
    # later identical instructions are fine). Burn that window on dummy
    # DoubleRow groups whose results are discarded (a token column is DMA'd
    # out so the instructions aren't dead-code-eliminated).
    if H_FP8 and WARM_GROUPS > 0:
        warm_sb = const.tile([P, WARM_GROUPS], F32, name="warm_sb")
        for g in range(WARM_GROUPS):
            wps = ps_h.tile([P, SB], F32, tag="ph")
            for c2 in range(EC2):
                nc.tensor.matmul(
                    wps[:],
                    w1_sb[:, 2 * c2 : 2 * c2 + 2, 0:P],
                    w1_sb[:, 2 * c2 : 2 * c2 + 2, 0:SB],
                    start=(c2 == 0),
                    stop=(c2 == EC2 - 1),
                    perf_mode=DR,
                )
            nc.vector.tensor_copy(warm_sb[:, g : g + 1], wps[:, 0:1])
        warm_d = dram.tile([P, WARM_GROUPS], F32, tag="warm_d")
        nc.sync.dma_start(warm_d[:], warm_sb[:])

    # Work deferred so the PE never waits on ScalarE output or DMA
    # roundtrips: flushed one (v/exp) or two (ctx) s-blocks later.
    pending_v = []
    pending_ctx = []

    def flush_one(queue):
        if queue:
            queue.pop(0)()

    for b in range(B):
        a_dram = dram.tile([1, S], SD, tag="a_dram")
        sums = onep.tile([1, NSB], F32, tag="sums")
        expT = work.tile([P, SK], SD, tag="expT")  # [p, k] = exp[k*128+p]
        recip = onep.tile([1, 1], F32, tag="recip")
        cstate = {}  # running DVE accumulator for the context reduction

        for sb in range(NSB):
            # encT[p, c, j] = enc[b, sb*512+j, c*128+p], from host transpose
            if b == 0 and sb == 0:
                encT = encT_first
            else:
                encT = encT_dma(b, sb)
            # ---- main matmuls: H^T chunks via fp8 DoubleRow, tanh(+q) ----
            # Each DoubleRow instruction contracts e-chunks (2*c2, 2*c2+1):
            # lhsT [128, 2, 128] and rhs [128, 2, 512] pair along dim 1.
            ths = []
            for cp in range(EC):
                ph = ps_h.tile([P, SB], F32, tag="ph")
                if H_FP8:
                    for c2 in range(EC2):
                        nc.tensor.matmul(
                            ph[:],
                            w1_sb[:, 2 * c2 : 2 * c2 + 2, ts(cp, P)],
                            encT[:, 2 * c2 : 2 * c2 + 2, :],
                            start=(c2 == 0),
                            stop=(c2 == EC2 - 1),
                            perf_mode=DR,
                        )
                else:
                    for c in range(EC):
                        nc.tensor.matmul(
                            ph[:],
                            w1_sb[:, c, ts(cp, P)],
                            encT[:, c, :],
                            start=(c == 0),
                            stop=(c == EC - 1),
                        )
                th = work.tile([P, SB], BF16, tag="tanh")
                nc.scalar.activation(
                    th[:],
                    ph[:],
                    Act.Tanh,
                    bias=qT[:, cp, b : b + 1],
                    scale=(1.0 / W1_SCALE) if H_FP8 else 1.0,
                )
                if DEBUG:
                    key = (b, sb, cp)
                    slot = {(0, 0, 0): 0, (0, 0, 7): 1, (0, 1, 0): 2,
                            (1, 0, 0): 3}.get(key)
                    if slot is not None:
                        nc.sync.dma_start(dbg["th"][:][slot], _f32(th[:]))
                    if key == (0, 0, 0):
                        phc = work.tile([P, SB], F32, tag="dbg_ph")
                        nc.vector.tensor_copy(phc[:], ph[:])
                        nc.sync.dma_start(dbg["ph"][:], phc[:])
                ths.append(th)

            if DEBUG and b == 0 and sb <= 1:
                nc.sync.dma_start(dbg["encT"][:][sb], encT[:])

            # prefetch the natural-layout bf16 enc chunks this block's
            # (2-block lagged) ctx reduction will need; issued after the
            # mains so they stay off the startup-critical DMA window
            cencs = []
            enc_b = enc[:][b].rearrange("(k p) e -> p k e", p=P)
            for k in range(sb * KSB, (sb + 1) * KSB):
                ce = cenc_pool.tile([P, E], CENC_DT, tag="cenc")
                nc.sync.dma_start(ce[:], enc_b[:, k, :])
                cencs.append(ce)

            flush_one(pending_v)
            if len(pending_ctx) >= 2:
                flush_one(pending_ctx)

            def make_v(
                b=b,
                sb=sb,
                ths=ths,
                sums=sums,
                expT=expT,
                recip=recip,
                a_dram=a_dram,
            ):
                def issue():
                    # scores[1, s] = sum_e v[e] * tanh[e, s] on the PE:
                    # vT chunk is a 1-column stationary (cheap ldweights),
                    # the tanh tiles stream as moving data; accumulate the
                    # 8 e-chunks in PSUM
                    pss = ps_s.tile([1, SB], F32, tag="pss", name="pss")
                    for cp in range(EC):
                        nc.tensor.matmul(
                            pss[:],
                            vT_b[:, cp : cp + 1],
                            ths[cp][:],
                            start=(cp == 0),
                            stop=(cp == EC - 1),
                        )
                    # exp + running sums (no max needed: |scores| <= 32)
                    exp_sb = onep.tile([1, SB], SD, tag="exp", name="exp_sb")
                    nc.scalar.activation(
                        exp_sb[:],
                        pss[:],
                        Act.Exp,
                        accum_out=sums[:, sb : sb + 1],
                    )
                    # transpose into expT[p, k] = exp[k*128+p] via a DRAM
                    # roundtrip (SBUF->SBUF partition-scatter DMA corrupts)
                    nc.sync.dma_start(a_dram[:, ts(sb, SB)], exp_sb[:])
                    nc.sync.dma_start(
                        expT[:, ts(sb, KSB)],
                        a_dram[:][0, ts(sb, SB)].rearrange(
                            "(k p) -> p k", p=P
                        ),
                    )
                    if DEBUG and sb == NSB - 1 and b <= 1:
                        nc.sync.dma_start(dbg["expT"][:][b], _f32(expT[:]))
                    if sb == NSB - 1:
                        # softmax denominator: must be issued AFTER the
                        # final sums write (Tile deps follow program order)
                        ssum = onep.tile([1, 1], F32, tag="ssum", name="ssum")
                        nc.vector.tensor_reduce(
                            ssum[:],
                            sums[:],
                            mybir.AxisListType.X,
                            mybir.AluOpType.add,
                        )
                        nc.vector.reciprocal(recip[:], ssum[:])

                return issue

            def make_ctx(
                b=b,
                sb=sb,
                cencs=cencs,
                expT=expT,
                cstate=cstate,
                recip=recip,
                last=(sb == NSB - 1),
            ):
                def issue():
                    # acc2[p, e] += enc[k*128+p, e] * attn[k*128+p] (VectorE)
                    # bf16 sub-chains (2x DVE rate) merged into an f32 main
                    # accumulator every 2 s-blocks - the 8-term bf16 partials
                    # stay tiny so their rounding is negligible;
                    # partition-sum via ones-matmul at the end
                    for j, k in enumerate(range(sb * KSB, (sb + 1) * KSB)):
                        nxt = accp.tile(
                            [P, E], SD, tag=f"cacc{k % 2}", name="cacc"
                        )
                        attn_k = _f32(expT[:, k : k + 1])
                        if k == 0:
                            nc.vector.tensor_scalar_mul(
                                nxt[:], cencs[j][:], attn_k
                            )
                        else:
                            nc.vector.scalar_tensor_tensor(
                                nxt[:],
                                cencs[j][:],
                                attn_k,
                                cstate["acc"][:],
                                mybir.AluOpType.mult,
                                mybir.AluOpType.add,
                            )
                        cstate["acc"] = nxt
                    if last:
                        acc2 = cstate["acc"]
                        for h in range(E // SB):
                            psc = ps_c.tile(
                                [1, SB], F32, tag="psc", name="psc"
                            )
                            nc.tensor.matmul(
                                psc[:],
                                ones_sd[:],
                                acc2[:, ts(h, SB)],
                                start=True,
                                stop=True,
                            )
                            ctx_sb = onep.tile(
                                [1, SB], F32, tag="ctx", name="ctx_sb"
                            )
                            nc.scalar.activation(
                                ctx_sb[:], psc[:], Act.Copy, scale=recip[:]
                            )
                            nc.sync.dma_start(
                                out[:][b : b + 1, ts(h, SB)], ctx_sb[:]
                            )

                return issue

            pending_v.append(make_v())
            pending_ctx.append(make_ctx())

    while pending_v or pending_ctx:
        flush_one(pending_v)
        flush_one(pending_ctx)


def build_nc():
    nc = bacc.Bacc(
        "TRN2", target_bir_lowering=False, debug=False, num_devices=N_CORES
    )
    enc = nc.dram_tensor("encoder_outputs", [B, S, E], CENC_DT, kind="ExternalInput")
    encT_d = nc.dram_tensor(
        "encoder_outputs_t", [B, E, S], U8 if H_FP8 else SD, kind="ExternalInput"
    )
    dec = nc.dram_tensor("decoder_output", [B, 1, E], Q_DT, kind="ExternalInput")
    w1 = nc.dram_tensor("w1", [E, E], U8 if H_FP8 else SD, kind="ExternalInput")
    b1 = nc.dram_tensor("b1", [E], F32, kind="ExternalInput")
    w2 = nc.dram_tensor("w2", [E, E], Q_DT, kind="ExternalInput")
    b2 = nc.dram_tensor("b2", [E], F32, kind="ExternalInput")
    v = nc.dram_tensor("v", [E, 1], SD, kind="ExternalInput")
    out = nc.dram_tensor("out", [B, E], F32, kind="ExternalOutput")
    dbg = {}
    if DEBUG:
        dbg["qT"] = nc.dram_tensor("dbg_qT", [P, EC, B], F32, kind="ExternalOutput")
        dbg["th"] = nc.dram_tensor("dbg_th", [4, P, SB], F32, kind="ExternalOutput")
        dbg["ph"] = nc.dram_tensor("dbg_ph", [P, SB], F32, kind="ExternalOutput")
        dbg["expT"] = nc.dram_tensor("dbg_expT", [2, P, SK], F32, kind="ExternalOutput")
        dbg["encT"] = nc.dram_tensor(
            "dbg_encT", [2, P, EC, SB], F8 if H_FP8 else SD,
            kind="ExternalOutput"
        )

    from contextlib import ExitStack

    with tile.TileContext(nc) as tc:
        with ExitStack() as ctx:
            _build_body(nc, tc, ctx, enc, encT_d, dec, w1, b1, w2, b2, v, out, dbg)
    nc.compile()
    return nc


_NC_CACHE = None


def _get_nc():
    global _NC_CACHE
    if _NC_CACHE is None:
        _NC_CACHE = build_nc()
    return _NC_CACHE


def make_in_maps(inputs):
    """Host-side prep: shard over batch, quantize (fp8 transposed enc for the
    H matmul, bf16 natural enc for the context stage, fp8 w1 scaled by 64)."""
    f32 = np.float32
    q_np = mybir.dt.np(Q_DT)
    cenc_np = mybir.dt.np(CENC_DT)
    h_np = mybir.dt.np(F8) if H_FP8 else f32
    enc_all = np.asarray(inputs["encoder_outputs"], dtype=f32)
    enc_bf16 = np.ascontiguousarray(enc_all.astype(cenc_np))
    encT_f8 = np.ascontiguousarray(
        enc_all.astype(h_np).transpose(0, 2, 1)
    )
    dec_bf16 = np.asarray(inputs["decoder_output"], dtype=f32).astype(q_np)
    w1_f32 = np.asarray(inputs["w1"], dtype=f32)
    w1_f8 = (w1_f32 * f32(W1_SCALE)).astype(h_np) if H_FP8 else w1_f32
    if H_FP8:
        encT_f8 = encT_f8.view(np.uint8)
        w1_f8 = w1_f8.view(np.uint8)
    w2_bf16 = np.asarray(inputs["w2"], dtype=f32).astype(q_np)
    in_maps = []
    for i in range(N_CORES):
        sl = slice(i * B, (i + 1) * B)
        in_maps.append(
            {
                "encoder_outputs": np.ascontiguousarray(enc_bf16[sl]),
                "encoder_outputs_t": encT_f8[sl],
                "decoder_output": np.ascontiguousarray(dec_bf16[sl]),
                "w1": w1_f8,
                "b1": np.ascontiguousarray(inputs["b1"], dtype=f32),
                "w2": w2_bf16,
                "b2": np.ascontiguousarray(inputs["b2"], dtype=f32),
                "v": np.ascontiguousarray(inputs["v"], dtype=f32),
            }
        )
    return in_maps


def run(inputs, trace=False):
    """Run on hardware. Returns (output [32, 1024] f32, exec_time_ns or None)."""
    nc = _get_nc()
    in_maps = make_in_maps(inputs)
    res = run_bass_kernel_spmd(
        nc, in_maps, core_ids=list(range(N_CORES)), trace=trace
    )
    out = np.concatenate([np.asarray(r["out"]) for r in res.results], axis=0)
    return out, res.exec_time_ns


def kernel(**inputs):
    out, _ = run(inputs)
    return out
